# revision 11
# baseline (speedup 1.0000x reference)
"""Multi-head attention + residual + LayerNorm on 8 Trainium2 NeuronCores.

Reference computation (B=2, S=2048, D=1024, H=16, HD=64):
    q,k,v = split_heads(x@Wq+bq), ...       # [B,H,S,HD]
    attn  = softmax(q k^T / sqrt(HD))
    out   = (attn v) merged -> [B,S,D] @ Wp + bp
    y     = LayerNorm(x + out) * gamma + beta

Sharding: 8 cores = 2 batches x 4 query-slices of 512 rows.
Each core computes QKV projections for its 512-row slice.  Scores+exp
for the core's OWN 512 keys are computed during phase 1 straight from
the local K^T (no communication), which keeps the scalar engine (the
exp bottleneck) busy while K^T/V slices are AllGathered in 4 grouped
collectives.  Remote key slices are consumed in per-core rotated order
(own, r+1, r+2, r+3) -- softmax is key-permutation invariant -- with
the rank-dependent gather-output offsets supplied via a per-core
`perm` input and register-offset APs.

Numerics: attention path in bf16 (matmuls accumulate f32 in PSUM),
residual + LayerNorm in f32.
"""

import os

import ml_dtypes
import numpy as np

import concourse.bacc as bacc
import concourse.tile as tile
from concourse import mybir
from concourse.bass_utils import run_bass_kernel_spmd

B, S, D, H, HD = 2, 2048, 1024, 16, 64
EPS = 1e-5
NCORES = 8
SL = S // 4          # 512 query rows per core
GROUPS = [[0, 1, 2, 3], [4, 5, 6, 7]]
# hp groups per collective: fire small gathers first so hp0's remote
# keys land as soon as the collective stack is warm.
GATHER_GROUPS = [[0], [1], [2, 3], [4, 5, 6, 7]]
BF = mybir.dt.bfloat16
F32 = mybir.dt.float32
I32 = mybir.dt.int32
Act = mybir.ActivationFunctionType
Alu = mybir.AluOpType

KBLK = 128 * SL      # elements in one K^T (or V) block per head pair


def build_program():
    nc = bacc.Bacc("TRN2", target_bir_lowering=False, debug=False,
                   num_devices=NCORES)

    # ---- I/O ----
    xT_d = nc.dram_tensor("xT", [D, SL], BF, kind="ExternalInput")
    xq_d = nc.dram_tensor("xq", [SL, D], F32, kind="ExternalInput")
    wq_d = nc.dram_tensor("wq", [D, D], BF, kind="ExternalInput")
    wk_d = nc.dram_tensor("wk", [D, D], BF, kind="ExternalInput")
    wv_d = nc.dram_tensor("wv", [D, D], BF, kind="ExternalInput")
    wp_d = nc.dram_tensor("wp", [D, D], BF, kind="ExternalInput")
    bq_d = nc.dram_tensor("bq", [D], F32, kind="ExternalInput")
    bk_d = nc.dram_tensor("bk", [D], F32, kind="ExternalInput")
    bv_d = nc.dram_tensor("bv", [D], F32, kind="ExternalInput")
    bp_d = nc.dram_tensor("bp", [D], BF, kind="ExternalInput")
    gamma_d = nc.dram_tensor("gamma", [D], F32, kind="ExternalInput")
    beta_d = nc.dram_tensor("beta", [D], F32, kind="ExternalInput")
    perm_d = nc.dram_tensor("perm", [4], I32, kind="ExternalInput")
    y_d = nc.dram_tensor("y", [SL, D], F32, kind="ExternalOutput")

    import concourse.bass as bass

    def bcast_ap(dram_t, parts=128):
        # replicate a [D] dram vector across `parts` partitions
        return bass.AP(tensor=dram_t, offset=0, ap=[[0, parts], [1, D]])

    with tile.TileContext(nc) as tc:
        with (
            tc.tile_pool(name="persist", bufs=1) as persist,
            tc.tile_pool(name="dram", bufs=1, space="DRAM") as dram,
        ):
            # persistent tiles
            qt_sb = persist.tile([128, 8, SL], BF)        # Q^T (bias added)
            ktloc = persist.tile([128, 8, SL], BF)        # local K^T (bias added)
            exp_own = persist.tile([128, 8, 4, 2, SL], BF)  # exp of own-key scores
            wp_sb = persist.tile([128, 8, 2, 512], BF)
            outT_sb = persist.tile([128, 8, SL], BF)      # normalized attn out^T
            bv_bc = persist.tile([128, D], F32)
            gamma_bc = persist.tile([128, D], F32)
            beta_bc = persist.tile([128, D], F32)
            bq_sb = persist.tile([128, 8], F32)
            bk_sb = persist.tile([128, 8], F32)
            bp_sb = persist.tile([1, D], BF)
            ones_sb = persist.tile([1, 128], BF)
            eps_sb = persist.tile([128, 1], F32)
            perm_sb = persist.tile([1, 4], I32)
            warm_sb = persist.tile([1, 2], F32)
            # manually double-buffered V tiles, ones columns memset once:
            # layout [V_A | 1 | V_B | 1] per key chunk
            vh_tiles = [persist.tile([128, 16, 130], BF, name=f"vh{i}") for i in range(2)]

            # DRAM scratch: per gather-group bounce/gather buffers.
            # Per hp: block 0 = K^T rows [hp*128,(hp+1)*128) flat [p, s]
            # (p-major); block 1 = V cols [hp*128,(hp+1)*128) flat [s, c].
            kvb_g = [dram.tile([len(g), 2, KBLK], BF, name=f"kvb{i}")
                     for i, g in enumerate(GATHER_GROUPS)]
            kvg_g = [dram.tile([4, len(g), 2, KBLK], BF, name=f"kvg{i}")
                     for i, g in enumerate(GATHER_GROUPS)]
            dumb_in = dram.tile([1, 128], BF, name="dumb_in")
            dumb_out = dram.tile([4, 128], BF, name="dumb_out")
            rb_d = dram.tile([8, 1024], BF, name="rb_d")  # per-hp recip bounce
            zro_sb = persist.tile([1, 128], BF)
            nc.vector.memset(zro_sb[:], 0.0)
            nc.gpsimd.dma_start(dumb_in[:], zro_sb[:])
            nc.gpsimd.collective_compute(
                "AllGather", Alu.bypass, replica_groups=GROUPS,
                ins=[dumb_in[:].opt()], outs=[dumb_out[:].opt()])

            # warm the ACT table set (ln+exp live in one set) ASAP
            nc.vector.memset(warm_sb[:], 1.0)
            nc.scalar.activation(warm_sb[0:1, 0:1], warm_sb[0:1, 1:2],
                                 Act.Exp, scale=1.0)

            # small/early loads (biases feed phase-1 epilogues)
            nc.sync.dma_start(perm_sb[:], perm_d.ap().rearrange("(o n) -> o n", o=1))
            nc.sync.dma_start(bk_sb[:], bk_d.ap().rearrange("(co p) -> p co", p=128))
            nc.sync.dma_start(bq_sb[:], bq_d.ap().rearrange("(co p) -> p co", p=128))
            nc.sync.dma_start(bv_bc[:], bcast_ap(bv_d))
            nc.vector.memset(ones_sb[:], 1.0)
            nc.vector.memset(eps_sb[:], EPS)
            for t in vh_tiles:
                nc.vector.memset(t[:, :, 64:65], 1.0)
                nc.vector.memset(t[:, :, 129:130], 1.0)

            # permuted gather-slice indices (j_n = (rank+n)%4), as registers
            # raw reg_load + snap (value_load's bounds asserts are not
            # supported by this runtime and crash the device)
            jv = []
            for n in range(4):
                _r = nc.gpsimd.alloc_register(f"jv{n}")
                nc.gpsimd.reg_load(_r, perm_sb[0:1, n:n + 1])
                jv.append(nc.gpsimd.snap(_r, donate=True))

            hp2g = {}
            for gi, g in enumerate(GATHER_GROUPS):
                for li, hp in enumerate(g):
                    hp2g[hp] = (gi, li)

            def kvg_k_ap(hp, n):
                """K^T block of gather slice j_n for head pair hp: [128, 4, 128]."""
                gi, li = hp2g[hp]
                js = len(GATHER_GROUPS[gi]) * 2 * KBLK
                static = li * 2 * KBLK
                return bass.AP(tensor=kvg_g[gi][:].tensor, offset=jv[n] * js + static,
                               ap=[[SL, 128], [128, 4], [1, 128]],
                               dep_tracking_offset=static)

            def kvg_v_ap(hp, n, chalf):
                """V block [s,c] of slice j_n for hp, c half: [128, 4, 64]."""
                gi, li = hp2g[hp]
                js = len(GATHER_GROUPS[gi]) * 2 * KBLK
                static = li * 2 * KBLK + KBLK + chalf * 64
                return bass.AP(tensor=kvg_g[gi][:].tensor, offset=jv[n] * js + static,
                               ap=[[128, 128], [128 * 128, 4], [1, 64]],
                               dep_tracking_offset=static)

            # ---------------- phase 1 + own-key scores ----------------
            with (
                tc.tile_pool(name="ph1w", bufs=1) as ph1w,
                tc.tile_pool(name="ph1", bufs=3) as ph1,
                tc.tile_pool(name="psum1", bufs=4, space="PSUM") as psum1,
                tc.tile_pool(name="psc1", bufs=2, space="PSUM") as psc1,
            ):
                # input loads: x^T first, then weights row-chunks (full
                # [128,1024] rows: 2KB/partition lines -> big DMA packets)
                xt_c, wk_c, wq_c, wv_c = [], [], [], []
                for ci in range(8):
                    xt = ph1w.tile([128, SL], BF, tag=f"xt{ci}")
                    nc.sync.dma_start(xt[:], xT_d[ci * 128:(ci + 1) * 128, :])
                    xt_c.append(xt)
                for ci in range(8):
                    wk = ph1w.tile([128, D], BF, tag=f"wk{ci}")
                    nc.sync.dma_start(wk[:], wk_d[ci * 128:(ci + 1) * 128, :])
                    wk_c.append(wk)
                    wq = ph1w.tile([128, D], BF, tag=f"wq{ci}")
                    nc.sync.dma_start(wq[:], wq_d[ci * 128:(ci + 1) * 128, :])
                    wq_c.append(wq)
                for ci in range(8):
                    wv = ph1w.tile([128, D], BF, tag=f"wv{ci}")
                    nc.sync.dma_start(wv[:], wv_d[ci * 128:(ci + 1) * 128, :])
                    wv_c.append(wv)
                # loads needed later (phase 2/3) queue behind on sync
                nc.sync.dma_start(wp_sb[:], wp_d.ap().rearrange(
                    "(hp p) (dh q) -> p hp dh q", p=128, q=512))
                nc.sync.dma_start(gamma_bc[:], bcast_ap(gamma_d))
                nc.sync.dma_start(beta_bc[:], bcast_ap(beta_d))
                nc.sync.dma_start(bp_sb[:], bp_d.ap().rearrange("(o d) -> o d", o=1))

                def k_chunk(co):
                    ps = psum1.tile([128, SL], F32, tag="ps1", name=f"psk{co}")
                    for ci in range(8):
                        nc.tensor.matmul(ps[:], wk_c[ci][:, co * 128:(co + 1) * 128],
                                         xt_c[ci][:], start=(ci == 0), stop=(ci == 7))
                    nc.vector.tensor_scalar_add(ktloc[:, co, :], ps[:],
                                                bk_sb[:, co:co + 1])
                    gi, li = hp2g[co]
                    dst = kvb_g[gi][li, 0, :].rearrange("(p s) -> p s", p=128)
                    nc.gpsimd.dma_start(dst, ktloc[:, co, :])

                def q_chunk(co):
                    ps = psum1.tile([128, SL], F32, tag="ps1", name=f"psq{co}")
                    for ci in range(8):
                        nc.tensor.matmul(ps[:], wq_c[ci][:, co * 128:(co + 1) * 128],
                                         xt_c[ci][:], start=(ci == 0), stop=(ci == 7))
                    nc.vector.tensor_scalar_add(qt_sb[:, co, :], ps[:],
                                                bq_sb[:, co:co + 1])

                def sc_own(hp, kcl):
                    """scores+exp for own key chunk kcl (128 keys), both heads."""
                    ps = psc1.tile([128, 2, SL], F32, tag="sc1", name=f"so{hp}{kcl}")
                    nc.tensor.matmul(ps[:, 0, :], ktloc[0:64, hp, kcl * 128:(kcl + 1) * 128],
                                     qt_sb[0:64, hp, :], start=True, stop=True)
                    nc.tensor.matmul(ps[:, 1, :], ktloc[64:128, hp, kcl * 128:(kcl + 1) * 128],
                                     qt_sb[64:128, hp, :], start=True, stop=True)
                    nc.scalar.activation(exp_own[:, hp, kcl, :, :], ps[:],
                                         Act.Exp, scale=0.125)

                def v_block(cb):
                    """V columns [cb*128,(cb+1)*128) for all 512 own rows."""
                    gi, li = hp2g[cb]
                    for sc in range(4):
                        psw = psum1.tile([128, 512], F32, tag="ps1", name=f"pv{cb}{sc}")
                        ps = psw[:, 0:128]
                        for ci in range(8):
                            nc.tensor.matmul(ps, xt_c[ci][:, sc * 128:(sc + 1) * 128],
                                             wv_c[ci][:, cb * 128:(cb + 1) * 128],
                                             start=(ci == 0), stop=(ci == 7))
                        v_t = ph1.tile([128, 128], BF, tag="vt", name=f"vt{cb}{sc}")
                        nc.vector.tensor_add(v_t[:], ps,
                                             bv_bc[:, cb * 128:(cb + 1) * 128])
                        dst = kvb_g[gi][li, 1, :].rearrange(
                            "(sc p c) -> sc p c", sc=4, p=128)
                        nc.gpsimd.dma_start(dst[sc], v_t[:])

                def v_half2():
                    """V columns 512:1024 (hp 4-7), amortized N=512."""
                    for sc in range(4):
                        ps = psum1.tile([128, 512], F32, tag="ps1", name=f"pvh{sc}")
                        for ci in range(8):
                            nc.tensor.matmul(ps[:], xt_c[ci][:, sc * 128:(sc + 1) * 128],
                                             wv_c[ci][:, 512:1024],
                                             start=(ci == 0), stop=(ci == 7))
                        v_t = ph1.tile([128, 512], BF, tag="vth", name=f"vth{sc}")
                        nc.vector.tensor_add(v_t[:], ps[:], bv_bc[:, 512:1024])
                        for cb in range(4, 8):
                            gi, li = hp2g[cb]
                            dst = kvb_g[gi][li, 1, :].rearrange(
                                "(sc p c) -> sc p c", sc=4, p=128)
                            nc.gpsimd.dma_start(dst[sc], v_t[:, (cb - 4) * 128:(cb - 3) * 128])

                def fire(gi):
                    nc.gpsimd.collective_compute(
                        "AllGather", Alu.bypass, replica_groups=GROUPS,
                        ins=[kvb_g[gi][:].opt()], outs=[kvg_g[gi][:].opt()])

                # PE order tuned so own-score chunks stream to ACT steadily
                # while K/V pieces feed the grouped gathers early.
                k_chunk(0); q_chunk(0)
                sc_own(0, 0); sc_own(0, 1)
                k_chunk(1); q_chunk(1)
                sc_own(0, 2); sc_own(0, 3)
                v_block(0); fire(0)
                sc_own(1, 0); sc_own(1, 1)
                v_block(1); fire(1)
                sc_own(1, 2); sc_own(1, 3)
                k_chunk(2); q_chunk(2)
                sc_own(2, 0); sc_own(2, 1)
                k_chunk(3); q_chunk(3)
                sc_own(2, 2); sc_own(2, 3)
                v_block(2)
                sc_own(3, 0); sc_own(3, 1)
                v_block(3); fire(2)
                sc_own(3, 2); sc_own(3, 3)
                k_chunk(4); q_chunk(4)
                sc_own(4, 0); sc_own(4, 1)
                k_chunk(5); q_chunk(5)
                sc_own(4, 2); sc_own(4, 3)
                k_chunk(6); q_chunk(6)
                sc_own(5, 0); sc_own(5, 1)
                k_chunk(7); q_chunk(7)
                sc_own(5, 2); sc_own(5, 3)
                v_half2(); fire(3)
                for hp in range(6, 8):
                    for kcl in range(4):
                        sc_own(hp, kcl)

            # ---------------- phase 2: remote scores + attention ----------------
            with (
                tc.tile_pool(name="kth", bufs=2) as kthp,
                tc.tile_pool(name="expp", bufs=2) as expp,
                tc.tile_pool(name="small", bufs=2) as small,
                tc.tile_pool(name="ps_sc", bufs=3, space="PSUM") as ps_sc,
                tc.tile_pool(name="ps_o", bufs=1, space="PSUM") as ps_o,
            ):
                def load_hp(hp):
                    kth_t = kthp.tile([128, 12, 128], BF, tag="kth", name=f"kth{hp}")
                    for n in (1, 2, 3):
                        nc.gpsimd.dma_start(kth_t[:, 4 * (n - 1):4 * n, :], kvg_k_ap(hp, n))
                    vh_t = vh_tiles[hp % 2]
                    for n in range(4):
                        nc.gpsimd.dma_start(vh_t[:, 4 * n:4 * n + 4, 0:64], kvg_v_ap(hp, n, 0))
                        nc.gpsimd.dma_start(vh_t[:, 4 * n:4 * n + 4, 65:129], kvg_v_ap(hp, n, 1))
                    return kth_t, vh_t

                tiles = load_hp(0)
                for hp in range(8):
                    kth_t, vh_t = tiles
                    exp_r = expp.tile([128, 12, 2, SL], BF, tag="exp", name=f"er{hp}")
                    for s in range(12):
                        ps = ps_sc.tile([128, 2, SL], F32, tag="sc", name=f"sr{hp}{s}")
                        nc.tensor.matmul(ps[:, 0, :], kth_t[0:64, s, :],
                                         qt_sb[0:64, hp, :], start=True, stop=True)
                        nc.tensor.matmul(ps[:, 1, :], kth_t[64:128, s, :],
                                         qt_sb[64:128, hp, :], start=True, stop=True)
                        nc.scalar.activation(exp_r[:, s, :, :], ps[:], Act.Exp, scale=0.125)
                        if s == 5 and hp < 7:
                            tiles = load_hp(hp + 1)  # prefetch next head pair

                    oAB = ps_o.tile([65, 2, SL], F32, tag="oAB", name=f"o{hp}")
                    for s in range(16):
                        rhsA = (exp_own[:, hp, s, 0, :] if s < 4
                                else exp_r[:, s - 4, 0, :])
                        rhsB = (exp_own[:, hp, s, 1, :] if s < 4
                                else exp_r[:, s - 4, 1, :])
                        nc.tensor.matmul(oAB[:, 0, :], vh_t[:, s, 0:65], rhsA,
                                         start=(s == 0), stop=(s == 15))
                        nc.tensor.matmul(oAB[:, 1, :], vh_t[:, s, 65:130], rhsB,
                                         start=(s == 0), stop=(s == 15))

                    # softmax normalization: rows 0-63 = head dims, row 64 = sum
                    sAB = small.tile([1, 2, SL], F32, tag="sAB")
                    nc.vector.tensor_copy(sAB[:], oAB[64:65, :, :])
                    rABf = small.tile([1, 2, SL], F32, tag="rABf")
                    nc.vector.reciprocal_approx_fast(out=rABf[:], in_=sAB[:])
                    rAB = small.tile([1, 2, SL], BF, tag="rAB")
                    with nc.allow_low_precision("softmax scale in bf16"):
                        nc.vector.tensor_copy(rAB[:], rABf[:])
                    # broadcast the [1, 1024] reciprocal row to 64 partitions
                    # via a DRAM bounce (partition-stride-0 read)
                    nc.gpsimd.dma_start(rb_d[hp].rearrange("(o n) -> o n", o=1), rAB[:])
                    rbb = small.tile([64, 2, SL], BF, tag="rbb")
                    nc.gpsimd.dma_start(
                        rbb[:], bass.AP(tensor=rb_d[:].tensor, offset=hp * 1024,
                                        ap=[[0, 64], [512, 2], [1, 512]]))
                    nc.vector.tensor_mul(outT_sb[0:64, hp, :], oAB[0:64, 0, :],
                                         rbb[:, 0, :])
                    tmpB = small.tile([64, SL], BF, tag="tmpB")
                    nc.vector.tensor_mul(tmpB[:], oAB[0:64, 1, :], rbb[:, 1, :])
                    nc.gpsimd.dma_start(outT_sb[64:128, hp, :], tmpB[:])

            # ---------------- phase 3: out-projection + residual + LayerNorm ----------------
            with (
                tc.tile_pool(name="ph3", bufs=3) as ph3,
                tc.tile_pool(name="ph3s", bufs=4) as ph3s,
                tc.tile_pool(name="xqp", bufs=2) as xqp,
                tc.tile_pool(name="psum3", bufs=4, space="PSUM") as psum3,
            ):
                xq_t = []
                for qc in range(4):
                    t = xqp.tile([128, D], F32, tag=f"xq{qc % 2}", name=f"xq{qc}")
                    nc.sync.dma_start(t[:], xq_d[qc * 128:(qc + 1) * 128, :])
                    xq_t.append(t)
                for qc in range(4):
                    y_t = ph3.tile([128, D], F32, tag="y")
                    for dh in range(2):
                        ps = psum3.tile([128, 512], F32, tag="py")
                        for hp in range(8):
                            nc.tensor.matmul(ps[:], outT_sb[:, hp, qc * 128:(qc + 1) * 128],
                                             wp_sb[:, hp, dh, :],
                                             start=(hp == 0), stop=False)
                        # + bp via a rank-1 matmul with a ones row
                        nc.tensor.matmul(ps[:], ones_sb[0:1, :],
                                         bp_sb[0:1, dh * 512:(dh + 1) * 512],
                                         start=False, stop=True)
                        nc.vector.tensor_add(y_t[:, dh * 512:(dh + 1) * 512], ps[:],
                                             xq_t[qc][:, dh * 512:(dh + 1) * 512])
                    # LayerNorm over D=1024
                    stats = ph3s.tile([128, 2, 6], F32, tag="stats")
                    nc.vector.bn_stats(stats[:, 0, :], y_t[:, 0:512])
                    nc.vector.bn_stats(stats[:, 1, :], y_t[:, 512:1024])
                    mv = ph3s.tile([128, 2], F32, tag="mv")
                    nc.vector.bn_aggr(mv[:], stats[:])
                    # rstd = exp(-0.5*ln(var+eps)) -- stays in the ln/exp table set
                    lnv = ph3s.tile([128, 1], F32, tag="lnv")
                    nc.scalar.activation(lnv[:], mv[:, 1:2], Act.Ln, bias=eps_sb[:])
                    rstd = ph3s.tile([128, 1], F32, tag="rstd")
                    nc.scalar.activation(rstd[:], lnv[:], Act.Exp, scale=-0.5)
                    # y = ((y - mu) * gamma) * rstd + beta
                    nc.vector.scalar_tensor_tensor(
                        out=y_t[:], in0=y_t[:], scalar=mv[:, 0:1], in1=gamma_bc[:],
                        op0=Alu.subtract, op1=Alu.mult)
                    nc.vector.scalar_tensor_tensor(
                        out=y_t[:], in0=y_t[:], scalar=rstd[:], in1=beta_bc[:],
                        op0=Alu.mult, op1=Alu.add)
                    nc.sync.dma_start(y_d[qc * 128:(qc + 1) * 128, :], y_t[:])

    nc.compile()
    return nc


_PROGRAM = None


def _get_program():
    global _PROGRAM
    if _PROGRAM is None:
        _PROGRAM = build_program()
    return _PROGRAM


def kernel(**inputs):
    x = np.asarray(inputs["x"], np.float32)
    bf = ml_dtypes.bfloat16
    shared = {
        "wq": np.asarray(inputs["Wq"], np.float32).astype(bf),
        "wk": np.asarray(inputs["Wk"], np.float32).astype(bf),
        "wv": np.asarray(inputs["Wv"], np.float32).astype(bf),
        "wp": np.asarray(inputs["Wp"], np.float32).astype(bf),
        "bq": np.asarray(inputs["bq"], np.float32),
        "bk": np.asarray(inputs["bk"], np.float32),
        "bv": np.asarray(inputs["bv"], np.float32),
        "bp": np.asarray(inputs["bp"], np.float32).astype(bf),
        "gamma": np.asarray(inputs["gamma"], np.float32),
        "beta": np.asarray(inputs["beta"], np.float32),
    }
    in_maps = []
    for c in range(NCORES):
        b, i = c // 4, c % 4
        xs = np.ascontiguousarray(x[b, i * SL:(i + 1) * SL, :])
        m = dict(shared)
        m["xT"] = np.ascontiguousarray(xs.T).astype(bf)
        m["xq"] = xs
        m["perm"] = np.array([(i + n) % 4 for n in range(4)], np.int32)
        in_maps.append(m)

    nc = _get_program()
    trace_dir = os.environ.get("BASS_KERNEL_TRACE_DIR")
    kwargs = {}
    if trace_dir:
        kwargs = {"trace": True, "tmpdir": trace_dir}
    res = run_bass_kernel_spmd(nc, in_maps, core_ids=list(range(NCORES)), **kwargs)

    out = np.empty((B, S, D), np.float32)
    for c in range(NCORES):
        b, i = c // 4, c % 4
        out[b, i * SL:(i + 1) * SL, :] = res.results[c]["y"]
    if trace_dir:
        kernel.last_exec_time_ns = res.exec_time_ns
        kernel.last_trace = res.instructions_and_trace
    return out


# revision 14
# speedup vs baseline: 1.0078x; 1.0078x over previous
"""Multi-head attention + residual + LayerNorm on 8 Trainium2 NeuronCores.

Reference computation (B=2, S=2048, D=1024, H=16, HD=64):
    q,k,v = split_heads(x@Wq+bq), ...       # [B,H,S,HD]
    attn  = softmax(q k^T / sqrt(HD))
    out   = (attn v) merged -> [B,S,D] @ Wp + bp
    y     = LayerNorm(x + out) * gamma + beta

Sharding: 8 cores = 2 batches x 4 query-slices of 512 rows.
Each core computes QKV projections for its 512-row slice.  Scores+exp
for the core's OWN 512 keys are computed during phase 1 straight from
the local K^T (no communication), which keeps the scalar engine (the
exp bottleneck) busy while K^T/V slices are AllGathered in 4 grouped
collectives.  Remote key slices are consumed in per-core rotated order
(own, r+1, r+2, r+3) -- softmax is key-permutation invariant -- with
the rank-dependent gather-output offsets supplied via a per-core
`perm` input and register-offset APs.

Numerics: attention path in bf16 (matmuls accumulate f32 in PSUM),
residual + LayerNorm in f32.
"""

import os

import ml_dtypes
import numpy as np

import concourse.bacc as bacc
import concourse.tile as tile
from concourse import mybir
from concourse.bass_utils import run_bass_kernel_spmd

B, S, D, H, HD = 2, 2048, 1024, 16, 64
EPS = 1e-5
NCORES = 8
SL = S // 4          # 512 query rows per core
GROUPS = [[0, 1, 2, 3], [4, 5, 6, 7]]
# hp groups per collective: fire small gathers first so hp0's remote
# keys land as soon as the collective stack is warm.
GATHER_GROUPS = [[0], [1], [2, 3], [4, 5, 6, 7]]
BF = mybir.dt.bfloat16
F32 = mybir.dt.float32
I32 = mybir.dt.int32
Act = mybir.ActivationFunctionType
Alu = mybir.AluOpType

KBLK = 128 * SL      # elements in one K^T (or V) block per head pair


def build_program():
    nc = bacc.Bacc("TRN2", target_bir_lowering=False, debug=False,
                   num_devices=NCORES)

    # ---- I/O ----
    xT_d = nc.dram_tensor("xT", [D, SL], BF, kind="ExternalInput")
    xq_d = nc.dram_tensor("xq", [SL, D], F32, kind="ExternalInput")
    wq_d = nc.dram_tensor("wq", [D, D], BF, kind="ExternalInput")
    wk_d = nc.dram_tensor("wk", [D, D], BF, kind="ExternalInput")
    wv_d = nc.dram_tensor("wv", [D, D], BF, kind="ExternalInput")
    wp_d = nc.dram_tensor("wp", [D, D], BF, kind="ExternalInput")
    bq_d = nc.dram_tensor("bq", [D], F32, kind="ExternalInput")
    bk_d = nc.dram_tensor("bk", [D], F32, kind="ExternalInput")
    bv_d = nc.dram_tensor("bv", [D], F32, kind="ExternalInput")
    bp_d = nc.dram_tensor("bp", [D], BF, kind="ExternalInput")
    gamma_d = nc.dram_tensor("gamma", [D], F32, kind="ExternalInput")
    beta_d = nc.dram_tensor("beta", [D], F32, kind="ExternalInput")
    perm_d = nc.dram_tensor("perm", [4], I32, kind="ExternalInput")
    y_d = nc.dram_tensor("y", [SL, D], F32, kind="ExternalOutput")

    import concourse.bass as bass

    def bcast_ap(dram_t, parts=128):
        # replicate a [D] dram vector across `parts` partitions
        return bass.AP(tensor=dram_t, offset=0, ap=[[0, parts], [1, D]])

    with tile.TileContext(nc) as tc:
        with (
            tc.tile_pool(name="persist", bufs=1) as persist,
            tc.tile_pool(name="dram", bufs=1, space="DRAM") as dram,
        ):
            # persistent tiles
            qt_sb = persist.tile([128, 8, SL], BF)        # Q^T (bias added)
            ktloc = persist.tile([128, 8, SL], BF)        # local K^T (bias added)
            exp_own = persist.tile([128, 8, 4, 2, SL], BF)  # exp of own-key scores
            wp_sb = persist.tile([128, 8, 2, 512], BF)
            outT_sb = persist.tile([128, 8, SL], BF)      # normalized attn out^T
            bv_bc = persist.tile([128, D], F32)
            gamma_bc = persist.tile([128, D], F32)
            beta_bc = persist.tile([128, D], F32)
            bq_sb = persist.tile([128, 8], F32)
            bk_sb = persist.tile([128, 8], F32)
            bp_sb = persist.tile([1, D], BF)
            ones_sb = persist.tile([1, 128], BF)
            eps_sb = persist.tile([128, 1], F32)
            perm_sb = persist.tile([1, 4], I32)
            warm_sb = persist.tile([1, 2], F32)
            # manually double-buffered V tiles, ones columns memset once:
            # layout [V_A | 1 | V_B | 1] per key chunk
            vh_tiles = [persist.tile([128, 16, 130], BF, name=f"vh{i}") for i in range(2)]

            # DRAM scratch: per gather-group bounce/gather buffers.
            # Per hp: block 0 = K^T rows [hp*128,(hp+1)*128) flat [p, s]
            # (p-major); block 1 = V cols [hp*128,(hp+1)*128) flat [s, c].
            kvb_g = [dram.tile([len(g), 2, KBLK], BF, name=f"kvb{i}")
                     for i, g in enumerate(GATHER_GROUPS)]
            kvg_g = [dram.tile([4, len(g), 2, KBLK], BF, name=f"kvg{i}")
                     for i, g in enumerate(GATHER_GROUPS)]
            dumb_in = dram.tile([1, 128], BF, name="dumb_in")
            dumb_out = dram.tile([4, 128], BF, name="dumb_out")
            rb_d = dram.tile([8, 1024], BF, name="rb_d")  # per-hp recip bounce
            zro_sb = persist.tile([1, 128], BF)
            nc.vector.memset(zro_sb[:], 0.0)
            nc.gpsimd.dma_start(dumb_in[:], zro_sb[:])
            nc.gpsimd.collective_compute(
                "AllGather", Alu.bypass, replica_groups=GROUPS,
                ins=[dumb_in[:].opt()], outs=[dumb_out[:].opt()])

            # warm the ACT table set (ln+exp live in one set) ASAP
            nc.vector.memset(warm_sb[:], 1.0)
            nc.scalar.activation(warm_sb[0:1, 0:1], warm_sb[0:1, 1:2],
                                 Act.Exp, scale=1.0)

            # small/early loads (biases feed phase-1 epilogues)
            nc.sync.dma_start(perm_sb[:], perm_d.ap().rearrange("(o n) -> o n", o=1))
            nc.sync.dma_start(bk_sb[:], bk_d.ap().rearrange("(co p) -> p co", p=128))
            nc.sync.dma_start(bq_sb[:], bq_d.ap().rearrange("(co p) -> p co", p=128))
            nc.scalar.dma_start(bv_bc[:], bcast_ap(bv_d))
            nc.vector.memset(ones_sb[:], 1.0)
            nc.vector.memset(eps_sb[:], EPS)
            for t in vh_tiles:
                nc.vector.memset(t[:, :, 64:65], 1.0)
                nc.vector.memset(t[:, :, 129:130], 1.0)

            # permuted gather-slice indices (j_n = (rank+n)%4), as registers
            # raw reg_load + snap (value_load's bounds asserts are not
            # supported by this runtime and crash the device)
            jv = []
            for n in range(4):
                _r = nc.gpsimd.alloc_register(f"jv{n}")
                nc.gpsimd.reg_load(_r, perm_sb[0:1, n:n + 1])
                jv.append(nc.gpsimd.snap(_r, donate=True))

            hp2g = {}
            for gi, g in enumerate(GATHER_GROUPS):
                for li, hp in enumerate(g):
                    hp2g[hp] = (gi, li)

            def kvg_k_ap(hp, n):
                """K^T block of gather slice j_n for head pair hp: [128, 4, 128]."""
                gi, li = hp2g[hp]
                js = len(GATHER_GROUPS[gi]) * 2 * KBLK
                static = li * 2 * KBLK
                return bass.AP(tensor=kvg_g[gi][:].tensor, offset=jv[n] * js + static,
                               ap=[[SL, 128], [128, 4], [1, 128]],
                               dep_tracking_offset=static)

            def kvg_v_ap(hp, n, chalf):
                """V block [s,c] of slice j_n for hp, c half: [128, 4, 64]."""
                gi, li = hp2g[hp]
                js = len(GATHER_GROUPS[gi]) * 2 * KBLK
                static = li * 2 * KBLK + KBLK + chalf * 64
                return bass.AP(tensor=kvg_g[gi][:].tensor, offset=jv[n] * js + static,
                               ap=[[128, 128], [128 * 128, 4], [1, 64]],
                               dep_tracking_offset=static)

            # ---------------- phase 1 + own-key scores ----------------
            with (
                tc.tile_pool(name="ph1w", bufs=1) as ph1w,
                tc.tile_pool(name="ph1", bufs=3) as ph1,
                tc.tile_pool(name="psum1", bufs=4, space="PSUM") as psum1,
                tc.tile_pool(name="psc1", bufs=2, space="PSUM") as psc1,
            ):
                # input loads, critical-path order: xt + wk0 + wq0 feed the
                # first K/Q chunks and thus the first own-score exps.
                # wv goes on the scalar queue (idle until the first exp).
                xt_c, wk_c, wq_c, wv_c = [], [], [], []
                for ci in range(8):
                    xt = ph1w.tile([128, SL], BF, tag=f"xt{ci}")
                    nc.sync.dma_start(xt[:], xT_d[ci * 128:(ci + 1) * 128, :])
                    xt_c.append(xt)
                for ci in range(8):
                    wk_c.append(ph1w.tile([128, D], BF, tag=f"wk{ci}", name=f"wk{ci}"))
                    wq_c.append(ph1w.tile([128, D], BF, tag=f"wq{ci}", name=f"wq{ci}"))
                    wv_c.append(ph1w.tile([128, D], BF, tag=f"wv{ci}", name=f"wv{ci}"))
                for ci in range(8):
                    nc.sync.dma_start(wk_c[ci][:], wk_d[ci * 128:(ci + 1) * 128, :])
                    nc.sync.dma_start(wq_c[ci][:], wq_d[ci * 128:(ci + 1) * 128, :])
                    nc.scalar.dma_start(wv_c[ci][:], wv_d[ci * 128:(ci + 1) * 128, :])
                # loads needed later (phase 2/3) queue behind on sync
                nc.sync.dma_start(wp_sb[:], wp_d.ap().rearrange(
                    "(hp p) (dh q) -> p hp dh q", p=128, q=512))
                nc.sync.dma_start(gamma_bc[:], bcast_ap(gamma_d))
                nc.sync.dma_start(beta_bc[:], bcast_ap(beta_d))
                nc.sync.dma_start(bp_sb[:], bp_d.ap().rearrange("(o d) -> o d", o=1))

                def k_chunk(co):
                    ps = psum1.tile([128, SL], F32, tag="ps1", name=f"psk{co}")
                    for ci in range(8):
                        nc.tensor.matmul(ps[:], wk_c[ci][:, co * 128:(co + 1) * 128],
                                         xt_c[ci][:], start=(ci == 0), stop=(ci == 7))
                    nc.vector.tensor_scalar_add(ktloc[:, co, :], ps[:],
                                                bk_sb[:, co:co + 1])
                    gi, li = hp2g[co]
                    dst = kvb_g[gi][li, 0, :].rearrange("(p s) -> p s", p=128)
                    nc.gpsimd.dma_start(dst, ktloc[:, co, :])

                def q_chunk(co):
                    ps = psum1.tile([128, SL], F32, tag="ps1", name=f"psq{co}")
                    for ci in range(8):
                        nc.tensor.matmul(ps[:], wq_c[ci][:, co * 128:(co + 1) * 128],
                                         xt_c[ci][:], start=(ci == 0), stop=(ci == 7))
                    nc.vector.tensor_scalar_add(qt_sb[:, co, :], ps[:],
                                                bq_sb[:, co:co + 1])

                def sc_own(hp, kcl):
                    """scores+exp for own key chunk kcl (128 keys), both heads."""
                    ps = psc1.tile([128, 2, SL], F32, tag="sc1", name=f"so{hp}{kcl}")
                    nc.tensor.matmul(ps[:, 0, :], ktloc[0:64, hp, kcl * 128:(kcl + 1) * 128],
                                     qt_sb[0:64, hp, :], start=True, stop=True)
                    nc.tensor.matmul(ps[:, 1, :], ktloc[64:128, hp, kcl * 128:(kcl + 1) * 128],
                                     qt_sb[64:128, hp, :], start=True, stop=True)
                    nc.scalar.activation(exp_own[:, hp, kcl, :, :], ps[:],
                                         Act.Exp, scale=0.125)

                def v_block(cb):
                    """V columns [cb*128,(cb+1)*128) for all 512 own rows."""
                    gi, li = hp2g[cb]
                    for sc in range(4):
                        psw = psum1.tile([128, 512], F32, tag="ps1", name=f"pv{cb}{sc}")
                        ps = psw[:, 0:128]
                        for ci in range(8):
                            nc.tensor.matmul(ps, xt_c[ci][:, sc * 128:(sc + 1) * 128],
                                             wv_c[ci][:, cb * 128:(cb + 1) * 128],
                                             start=(ci == 0), stop=(ci == 7))
                        v_t = ph1.tile([128, 128], BF, tag="vt", name=f"vt{cb}{sc}")
                        nc.vector.tensor_add(v_t[:], ps,
                                             bv_bc[:, cb * 128:(cb + 1) * 128])
                        dst = kvb_g[gi][li, 1, :].rearrange(
                            "(sc p c) -> sc p c", sc=4, p=128)
                        nc.gpsimd.dma_start(dst[sc], v_t[:])

                def v_half2():
                    """V columns 512:1024 (hp 4-7), amortized N=512."""
                    for sc in range(4):
                        ps = psum1.tile([128, 512], F32, tag="ps1", name=f"pvh{sc}")
                        for ci in range(8):
                            nc.tensor.matmul(ps[:], xt_c[ci][:, sc * 128:(sc + 1) * 128],
                                             wv_c[ci][:, 512:1024],
                                             start=(ci == 0), stop=(ci == 7))
                        v_t = ph1.tile([128, 512], BF, tag="vth", name=f"vth{sc}")
                        nc.vector.tensor_add(v_t[:], ps[:], bv_bc[:, 512:1024])
                        for cb in range(4, 8):
                            gi, li = hp2g[cb]
                            dst = kvb_g[gi][li, 1, :].rearrange(
                                "(sc p c) -> sc p c", sc=4, p=128)
                            nc.gpsimd.dma_start(dst[sc], v_t[:, (cb - 4) * 128:(cb - 3) * 128])

                def fire(gi):
                    nc.gpsimd.collective_compute(
                        "AllGather", Alu.bypass, replica_groups=GROUPS,
                        ins=[kvb_g[gi][:].opt()], outs=[kvg_g[gi][:].opt()])

                # PE order tuned so own-score chunks stream to ACT steadily
                # while K/V pieces feed the grouped gathers early.
                k_chunk(0); q_chunk(0)
                sc_own(0, 0); sc_own(0, 1)
                k_chunk(1); q_chunk(1)
                sc_own(0, 2); sc_own(0, 3)
                v_block(0); fire(0)
                sc_own(1, 0); sc_own(1, 1)
                v_block(1); fire(1)
                sc_own(1, 2); sc_own(1, 3)
                k_chunk(2); q_chunk(2)
                sc_own(2, 0); sc_own(2, 1)
                k_chunk(3); q_chunk(3)
                sc_own(2, 2); sc_own(2, 3)
                v_block(2)
                sc_own(3, 0); sc_own(3, 1)
                v_block(3); fire(2)
                sc_own(3, 2); sc_own(3, 3)
                k_chunk(4); q_chunk(4)
                sc_own(4, 0); sc_own(4, 1)
                k_chunk(5); q_chunk(5)
                sc_own(4, 2); sc_own(4, 3)
                k_chunk(6); q_chunk(6)
                sc_own(5, 0); sc_own(5, 1)
                k_chunk(7); q_chunk(7)
                sc_own(5, 2); sc_own(5, 3)
                v_half2(); fire(3)
                for hp in range(6, 8):
                    for kcl in range(4):
                        sc_own(hp, kcl)

            # ---------------- phase 2: remote scores + attention ----------------
            with (
                tc.tile_pool(name="kth", bufs=2) as kthp,
                tc.tile_pool(name="expp", bufs=2) as expp,
                tc.tile_pool(name="small", bufs=2) as small,
                tc.tile_pool(name="ps_sc", bufs=3, space="PSUM") as ps_sc,
                tc.tile_pool(name="ps_o", bufs=1, space="PSUM") as ps_o,
            ):
                def load_hp(hp):
                    kth_t = kthp.tile([128, 12, 128], BF, tag="kth", name=f"kth{hp}")
                    for n in (1, 2, 3):
                        nc.gpsimd.dma_start(kth_t[:, 4 * (n - 1):4 * n, :], kvg_k_ap(hp, n))
                    vh_t = vh_tiles[hp % 2]
                    for n in range(4):
                        nc.gpsimd.dma_start(vh_t[:, 4 * n:4 * n + 4, 0:64], kvg_v_ap(hp, n, 0))
                        nc.gpsimd.dma_start(vh_t[:, 4 * n:4 * n + 4, 65:129], kvg_v_ap(hp, n, 1))
                    return kth_t, vh_t

                tiles = load_hp(0)
                for hp in range(8):
                    kth_t, vh_t = tiles
                    exp_r = expp.tile([128, 12, 2, SL], BF, tag="exp", name=f"er{hp}")
                    for s in range(12):
                        ps = ps_sc.tile([128, 2, SL], F32, tag="sc", name=f"sr{hp}{s}")
                        nc.tensor.matmul(ps[:, 0, :], kth_t[0:64, s, :],
                                         qt_sb[0:64, hp, :], start=True, stop=True)
                        nc.tensor.matmul(ps[:, 1, :], kth_t[64:128, s, :],
                                         qt_sb[64:128, hp, :], start=True, stop=True)
                        nc.scalar.activation(exp_r[:, s, :, :], ps[:], Act.Exp, scale=0.125)
                        if s == 5 and hp < 7:
                            tiles = load_hp(hp + 1)  # prefetch next head pair

                    oAB = ps_o.tile([65, 2, SL], F32, tag="oAB", name=f"o{hp}")
                    for s in range(16):
                        rhsA = (exp_own[:, hp, s, 0, :] if s < 4
                                else exp_r[:, s - 4, 0, :])
                        rhsB = (exp_own[:, hp, s, 1, :] if s < 4
                                else exp_r[:, s - 4, 1, :])
                        nc.tensor.matmul(oAB[:, 0, :], vh_t[:, s, 0:65], rhsA,
                                         start=(s == 0), stop=(s == 15))
                        nc.tensor.matmul(oAB[:, 1, :], vh_t[:, s, 65:130], rhsB,
                                         start=(s == 0), stop=(s == 15))

                    # softmax normalization: rows 0-63 = head dims, row 64 = sum
                    sAB = small.tile([1, 2, SL], F32, tag="sAB")
                    nc.vector.tensor_copy(sAB[:], oAB[64:65, :, :])
                    rABf = small.tile([1, 2, SL], F32, tag="rABf")
                    nc.vector.reciprocal_approx_fast(out=rABf[:], in_=sAB[:])
                    rAB = small.tile([1, 2, SL], BF, tag="rAB")
                    with nc.allow_low_precision("softmax scale in bf16"):
                        nc.vector.tensor_copy(rAB[:], rABf[:])
                    # broadcast the [1, 1024] reciprocal row to 64 partitions
                    # via a DRAM bounce (partition-stride-0 read)
                    nc.gpsimd.dma_start(rb_d[hp].rearrange("(o n) -> o n", o=1), rAB[:])
                    rbb = small.tile([64, 2, SL], BF, tag="rbb")
                    nc.gpsimd.dma_start(
                        rbb[:], bass.AP(tensor=rb_d[:].tensor, offset=hp * 1024,
                                        ap=[[0, 64], [512, 2], [1, 512]]))
                    nc.vector.tensor_mul(outT_sb[0:64, hp, :], oAB[0:64, 0, :],
                                         rbb[:, 0, :])
                    tmpB = small.tile([64, SL], BF, tag="tmpB")
                    nc.vector.tensor_mul(tmpB[:], oAB[0:64, 1, :], rbb[:, 1, :])
                    nc.gpsimd.dma_start(outT_sb[64:128, hp, :], tmpB[:])

            # ---------------- phase 3: out-projection + residual + LayerNorm ----------------
            with (
                tc.tile_pool(name="ph3", bufs=3) as ph3,
                tc.tile_pool(name="ph3s", bufs=4) as ph3s,
                tc.tile_pool(name="xqp", bufs=2) as xqp,
                tc.tile_pool(name="psum3", bufs=4, space="PSUM") as psum3,
            ):
                xq_t = []
                for qc in range(4):
                    t = xqp.tile([128, D], F32, tag=f"xq{qc % 2}", name=f"xq{qc}")
                    nc.gpsimd.dma_start(t[:], xq_d[qc * 128:(qc + 1) * 128, :])
                    xq_t.append(t)
                for qc in range(4):
                    y_t = ph3.tile([128, D], F32, tag="y")
                    for dh in range(2):
                        ps = psum3.tile([128, 512], F32, tag="py")
                        for hp in range(8):
                            nc.tensor.matmul(ps[:], outT_sb[:, hp, qc * 128:(qc + 1) * 128],
                                             wp_sb[:, hp, dh, :],
                                             start=(hp == 0), stop=False)
                        # + bp via a rank-1 matmul with a ones row
                        nc.tensor.matmul(ps[:], ones_sb[0:1, :],
                                         bp_sb[0:1, dh * 512:(dh + 1) * 512],
                                         start=False, stop=True)
                        nc.vector.tensor_add(y_t[:, dh * 512:(dh + 1) * 512], ps[:],
                                             xq_t[qc][:, dh * 512:(dh + 1) * 512])
                    # LayerNorm over D=1024
                    stats = ph3s.tile([128, 2, 6], F32, tag="stats")
                    nc.vector.bn_stats(stats[:, 0, :], y_t[:, 0:512])
                    nc.vector.bn_stats(stats[:, 1, :], y_t[:, 512:1024])
                    mv = ph3s.tile([128, 2], F32, tag="mv")
                    nc.vector.bn_aggr(mv[:], stats[:])
                    # rstd = exp(-0.5*ln(var+eps)) -- stays in the ln/exp table set
                    lnv = ph3s.tile([128, 1], F32, tag="lnv")
                    nc.scalar.activation(lnv[:], mv[:, 1:2], Act.Ln, bias=eps_sb[:])
                    rstd = ph3s.tile([128, 1], F32, tag="rstd")
                    nc.scalar.activation(rstd[:], lnv[:], Act.Exp, scale=-0.5)
                    # y = ((y - mu) * gamma) * rstd + beta
                    nc.vector.scalar_tensor_tensor(
                        out=y_t[:], in0=y_t[:], scalar=mv[:, 0:1], in1=gamma_bc[:],
                        op0=Alu.subtract, op1=Alu.mult)
                    nc.vector.scalar_tensor_tensor(
                        out=y_t[:], in0=y_t[:], scalar=rstd[:], in1=beta_bc[:],
                        op0=Alu.mult, op1=Alu.add)
                    nc.gpsimd.dma_start(y_d[qc * 128:(qc + 1) * 128, :], y_t[:])

    nc.compile()
    return nc


_PROGRAM = None


def _get_program():
    global _PROGRAM
    if _PROGRAM is None:
        _PROGRAM = build_program()
    return _PROGRAM


def kernel(**inputs):
    x = np.asarray(inputs["x"], np.float32)
    bf = ml_dtypes.bfloat16
    shared = {
        "wq": np.asarray(inputs["Wq"], np.float32).astype(bf),
        "wk": np.asarray(inputs["Wk"], np.float32).astype(bf),
        "wv": np.asarray(inputs["Wv"], np.float32).astype(bf),
        "wp": np.asarray(inputs["Wp"], np.float32).astype(bf),
        "bq": np.asarray(inputs["bq"], np.float32),
        "bk": np.asarray(inputs["bk"], np.float32),
        "bv": np.asarray(inputs["bv"], np.float32),
        "bp": np.asarray(inputs["bp"], np.float32).astype(bf),
        "gamma": np.asarray(inputs["gamma"], np.float32),
        "beta": np.asarray(inputs["beta"], np.float32),
    }
    in_maps = []
    for c in range(NCORES):
        b, i = c // 4, c % 4
        xs = np.ascontiguousarray(x[b, i * SL:(i + 1) * SL, :])
        m = dict(shared)
        m["xT"] = np.ascontiguousarray(xs.T).astype(bf)
        m["xq"] = xs
        m["perm"] = np.array([(i + n) % 4 for n in range(4)], np.int32)
        in_maps.append(m)

    nc = _get_program()
    trace_dir = os.environ.get("BASS_KERNEL_TRACE_DIR")
    kwargs = {}
    if trace_dir:
        kwargs = {"trace": True, "tmpdir": trace_dir}
    res = run_bass_kernel_spmd(nc, in_maps, core_ids=list(range(NCORES)), **kwargs)

    out = np.empty((B, S, D), np.float32)
    for c in range(NCORES):
        b, i = c // 4, c % 4
        out[b, i * SL:(i + 1) * SL, :] = res.results[c]["y"]
    if trace_dir:
        kernel.last_exec_time_ns = res.exec_time_ns
        kernel.last_trace = res.instructions_and_trace
    return out
